# revision 9
# baseline (speedup 1.0000x reference)
"""Trainium2 Bass kernel for CustomSTFT (STFT -> mag/phase -> ISTFT roundtrip).

The roundtrip is linear in x, collapsing to a 7-tap polyphase band filter per
200-sample chunk:  out_chunk[c] = sum_{d=-3..3} K_d @ in_chunk[c+d].
Taps d=+-3 are dropped (tiny); the center tap d=0 runs in fp16; the four
off-center taps (d=+-1, +-2, block rowL2 <= 0.017 vs 0.75 for d=0) run in
fp8e4m3 with DoubleRow perf mode, pairing the two contraction halves of each
tap into a single K=200 matmul (2 fp8 MACs/cell/cycle).  Host pre-scales ph
by 1/16 and all weights by 16 so fp8 weights clear the e4m3 subnormal floor
and products need no rescale (measured rel err 1.05e-2 vs 2e-2 tolerance).
fp32 PSUM accumulate, fp16 output.  Boundary-chunk phantom-frame corrections
are applied on host.

The 200-wide contraction is split 128/72 (not 100/100) so the h0-plane input
DMA engages all 16 SDMA engines (engine <-> partition mapping), and per psum
group the fp16 matmuls are emitted for all 6 groups of a sample before the
fp8 ones, so the PE can start on fp16 data while fp8 weights/casts are still
in flight.

Sharding: batch 16 across 8 cores (2 samples each).
"""

import numpy as np

N_CORES = 8
B, T = 16, 240000
SPC = B // N_CORES          # samples per core
N_FFT, HOP, FREQ, PAD = 800, 200, 401, 400
C = HOP                     # chunk size 200
H = C // 2                  # 100: output row split (PSUM partition dim)
K0, K1 = 128, 72            # contraction split of the 200-sample chunk
NCH_XP = (T + 2 * PAD) // C     # 1204 chunks in edge-padded signal
NCH_P = NCH_XP + 2              # 1206 incl. one zero chunk each side
NCHP2 = 1216                    # padded chunk count (fp8 DR AP needs %16 steps)
NOUT = T // C                   # 1200 output chunks per sample
NTILE = 400                     # output chunks per matmul (psum free dim)
NT = NOUT // NTILE              # 3 interior tiles per sample
N_WARM = 6                      # dummy matmuls to cover until input DMA lands
DR_TAPS = (1, 2, 4, 5)          # off-center taps jp (d = jp-3), fp8 DoubleRow
RP = 112                        # padded r' stride in fp8 weights (%16 bytes)
SC = 16.0                       # host scaling: ph /= SC, weights *= SC

_cache = {}


def _host_weights():
    """Build device weight tensors + edge-correction blocks from scratch."""
    if "k16" in _cache:
        return _cache["k16"], _cache["k8"], _cache["em0"], _cache["em1"]
    import ml_dtypes
    n = np.arange(N_FFT)
    k = np.arange(FREQ)
    win = 0.5 * (1.0 - np.cos(2.0 * np.pi * np.arange(N_FFT) / N_FFT))
    angle = 2.0 * np.pi * np.outer(k, n) / N_FFT
    w_fwd_r = np.cos(angle) * win
    w_fwd_i = -np.sin(angle) * win
    inv_win = win / N_FFT
    w_bwd_r = np.cos(angle) * inv_win
    w_bwd_i = np.sin(angle) * inv_win
    M = w_bwd_r.T @ w_fwd_r - w_bwd_i.T @ w_fwd_i          # [800, 800] f64

    # Band kernels Kfull[r, (d+3)*C + s] = sum_i M[i*C+r, (i+d)*C+s]
    Kfull = np.zeros((C, 7 * C))
    for d in range(-3, 4):
        for i in range(4):
            j = i + d
            if 0 <= j <= 3:
                Kfull[:, (d + 3) * C:(d + 4) * C] += M[i * C:(i + 1) * C, j * C:(j + 1) * C]
    Ks = Kfull * SC
    ksplit = ((0, K0), (K0, C))   # (s start, s end) per contraction half

    # center tap fp16 weights: K16[s', 2*ho+h, r'] = Ks[ho*H+r', 3*C+h-half s']
    K16 = np.zeros((K0, 4, H), dtype=np.float16)
    for ho in range(2):
        for h, (s0, s1) in enumerate(ksplit):
            blk = Ks[ho * H:(ho + 1) * H, 3 * C + s0: 3 * C + s1]
            K16[0:s1 - s0, 2 * ho + h, :] = blk.T.astype(np.float16)

    # off-center tap fp8 weights: K8[s', ho, tapi, h, r'] (r' padded to RP)
    K8 = np.zeros((K0, 2, 4, 2, RP), dtype=ml_dtypes.float8_e4m3)
    for ho in range(2):
        for ti, jp in enumerate(DR_TAPS):
            for h, (s0, s1) in enumerate(ksplit):
                blk = Ks[ho * H:(ho + 1) * H, jp * C + s0: jp * C + s1]
                K8[0:s1 - s0, ho, ti, h, 0:H] = blk.T.astype(ml_dtypes.float8_e4m3)

    # host edge-correction blocks (subtract phantom-frame contributions)
    em0 = np.stack([M[3 * C:4 * C, (jj + 1) * C:(jj + 2) * C] for jj in range(3)])
    em1 = np.stack([M[0:C, jj * C:(jj + 1) * C] for jj in range(3)])
    _cache.update(k16=K16, k8=K8, em0=em0, em1=em1)
    return K16, K8, em0, em1


def _build_nc():
    if "nc" in _cache:
        return _cache["nc"]
    import concourse.mybir as mybir
    import concourse.tile as tile
    from concourse import bacc

    f32 = mybir.dt.float32
    f16 = mybir.dt.float16
    f8 = mybir.dt.float8e4
    DR = mybir.MatmulPerfMode.DoubleRow

    nc = bacc.Bacc("TRN2", target_bir_lowering=False, debug=False,
                   num_devices=N_CORES)
    ph_d = nc.dram_tensor("ph", [K0, SPC, 2, NCHP2], f16, kind="ExternalInput").ap()
    k16_d = nc.dram_tensor("k16", [K0, 4, H], f16, kind="ExternalInput").ap()
    k8_d = nc.dram_tensor("k8", [K0, 2, 4, 2, RP], f8, kind="ExternalInput").ap()
    out_d = nc.dram_tensor("out", [SPC, 2, H, NOUT], f16, kind="ExternalOutput").ap()

    with tile.TileContext(nc) as tc:
        with (
            tc.tile_pool(name="weights", bufs=1) as wpool,
            tc.tile_pool(name="data", bufs=1) as dpool,
            tc.tile_pool(name="outp", bufs=1) as opool,
            tc.tile_pool(name="pint", bufs=6, space="PSUM") as pint,
            tc.tile_pool(name="pwarm", bufs=1, space="PSUM") as pwarm,
        ):
            warm = wpool.tile([K0, 512], f16)
            k16 = wpool.tile([K0, 4, H], f16)
            k8 = wpool.tile([K0, 2, 4, 2, RP], f8)
            ph16 = dpool.tile([K0, SPC, 2, NCHP2], f16)
            ph8 = dpool.tile([K0, SPC, 2, NCHP2], f8)
            outs = opool.tile([H, SPC, 2, NOUT], f16)

            # warm-tile memset first so warmup matmuls can start immediately
            nc.vector.memset(warm[:], 0.0)
            # zero the h1-plane pad rows (72:128) the DMA never writes; the
            # full-plane fp8 casts read them (zero-weight lanes must stay
            # finite for the DR matmuls)
            # (start partition must be a multiple of 32; rows 64:72 are
            # re-written by the h1 DMA afterwards)
            nc.gpsimd.memset(ph16[64:K0, :, 1, :], 0.0)

            # --- input DMA, both HWDGE rings, in consumption order:
            # k16 + ss0-h0 first (f16 h0 matmuls), then ss0-h1, k8, ss1 ---
            nc.sync.dma_start(ph16[:, 0, 0, :], ph_d[:, 0, 0, :])
            nc.scalar.dma_start(k16[:], k16_d[:])
            nc.scalar.dma_start(ph16[0:K1, 0, 1, :], ph_d[0:K1, 0, 1, :])
            nc.sync.dma_start(ph16[:, 1, 0, :], ph_d[:, 1, 0, :])
            nc.scalar.dma_start(k8[:], k8_d[:])
            nc.sync.dma_start(ph16[0:K1, 1, 1, :], ph_d[0:K1, 1, 1, :])

            # --- PE warmup on zeroed scratch while input DMA is in flight ---
            wps = pwarm.tile([K0, 512], f32, tag="wps")
            for _ in range(N_WARM):
                nc.tensor.matmul(wps[:], warm[:, 0:K0], warm[:],
                                 start=True, stop=True)

            # --- fp16 -> fp8 casts (per sample): ss0 on DVE, ss1 on ACT so
            # neither blocks the psum copies ---
            nc.vector.tensor_copy(ph8[:, 0, :, :], ph16[:, 0, :, :])
            nc.scalar.copy(ph8[:, 1, :, :], ph16[:, 1, :, :])

            store_engs = [nc.sync, nc.scalar]
            nstore = 0
            for ss in range(SPC):
                pss = []
                # fp16 phase: center-tap matmuls for all 6 groups of this
                # sample, all h0 halves first — the h1-plane DMA and the fp8
                # weights/casts may still be in flight while h0 runs, and the
                # PE stream must stay gap-free or HAM re-throttles
                for ho in range(2):
                    for tidx in range(NT):
                        ps = pint.tile([H, NTILE], f32, tag="ps")
                        pss.append(ps)
                for h in range(2):
                    for ho in range(2):
                        for tidx in range(NT):
                            t0 = tidx * NTILE
                            ps = pss[ho * NT + tidx]
                            if h == 0:
                                nc.tensor.matmul(
                                    ps[:], k16[:, 2 * ho, :],
                                    ph16[:, ss, 0, t0 + 3: t0 + 3 + NTILE],
                                    start=True, stop=False)
                            else:
                                nc.tensor.matmul(
                                    ps[:], k16[0:K1, 2 * ho + 1, :],
                                    ph16[0:K1, ss, 1, t0 + 3: t0 + 3 + NTILE],
                                    start=False, stop=False)
                # fp8 DoubleRow phase + copy-out + store per group; copies
                # alternate DVE / ACT so psum drain never serializes on one
                # engine, and the very last group's copy+store is split in
                # half to shorten the end-of-kernel chain
                for ho in range(2):
                    for tidx in range(NT):
                        t0 = tidx * NTILE
                        ps = pss[ho * NT + tidx]
                        for ti, jp in enumerate(DR_TAPS):
                            nc.tensor.matmul(
                                ps[:],
                                k8[:, ho, ti, :, 0:H],
                                ph8[:, ss, :, t0 + jp: t0 + jp + NTILE],
                                start=False, stop=(ti == 3),
                                perf_mode=DR,
                            )
                        last = (ss == SPC - 1 and ho == 1 and tidx == NT - 1)
                        pieces = ((0, NTILE // 2), (NTILE // 2, NTILE)) if last \
                            else ((0, NTILE),)
                        for a, b in pieces:
                            ceng = nc.vector if nstore % 2 == 0 else nc.scalar
                            if ceng is nc.vector:
                                ceng.tensor_copy(outs[:, ss, ho, t0 + a:t0 + b],
                                                 ps[:, a:b])
                            else:
                                ceng.copy(outs[:, ss, ho, t0 + a:t0 + b],
                                          ps[:, a:b])
                            eng = store_engs[nstore % 2]
                            nstore += 1
                            eng.dma_start(out_d[ss, ho][:, t0 + a:t0 + b],
                                          outs[:, ss, ho, t0 + a:t0 + b])

    nc.compile()
    _cache["nc"] = nc
    return nc


def _host_inputs(x):
    """Marshal full input x [B, T] f32 into per-core input maps."""
    K16, K8, em0, em1 = _host_weights()
    xp = np.pad(x, ((0, 0), (PAD, PAD)), mode="edge")       # [B, 240800]
    P = np.pad(xp, ((0, 0), (C, C)))                        # [B, 241200]
    P16 = (P / SC).astype(np.float16)
    # Pc[b, c', s] = P16[b, c'*200 + s]
    Pc = P16.reshape(B, NCH_P, C)
    in_maps = []
    for core in range(N_CORES):
        ph = np.zeros((K0, SPC, 2, NCHP2), dtype=np.float16)
        for ss in range(SPC):
            b = core * SPC + ss
            # h0-plane: s' = s in [0,128); h1-plane: s' = s-128 in [0,72)
            ph[:, ss, 0, 0:NCH_P] = Pc[b, :, 0:K0].T
            ph[0:K1, ss, 1, 0:NCH_P] = Pc[b, :, K0:C].T
        in_maps.append({"ph": ph, "k16": K16, "k8": K8})
    return in_maps, xp, em0, em1


last_results = None  # BassKernelResults of the most recent run (for test harness)


def kernel(x, w_fwd_r=None, w_fwd_i=None, w_bwd_r=None, w_bwd_i=None):
    global last_results
    from concourse.bass_utils import run_bass_kernel_spmd

    x = np.asarray(x, dtype=np.float32)
    assert x.shape == (B, T), x.shape
    nc = _build_nc()
    in_maps, xp, em0, em1 = _host_inputs(x)

    res = run_bass_kernel_spmd(nc, in_maps, core_ids=list(range(N_CORES)))
    last_results = res

    y = np.empty((B, T), dtype=np.float32)
    for core in range(N_CORES):
        od = res.results[core]["out"]                       # [SPC,2,H,NOUT] f16
        for ss in range(SPC):
            # y[t*200 + ho*100 + r'] = od[ss, ho, r', t]
            y[core * SPC + ss] = od[ss].astype(np.float32).transpose(2, 0, 1).reshape(T)

    # host edge corrections: first/last output chunk miss one phantom frame
    xpc = xp.astype(np.float64).reshape(B, NCH_XP, C)
    y[:, 0:C] -= np.einsum('jrs,bjs->br', em0, xpc[:, 0:3]).astype(np.float32)
    y[:, T - C:T] -= np.einsum('jrs,bjs->br', em1, xpc[:, 1201:1204]).astype(np.float32)
    return y
